# revision 16
# baseline (speedup 1.0000x reference)
"""Trainium2 Bass kernel for nn_ConvTokMWLA: conv tokenizer + 3 encoder layers of
multi-window (per-head banded) local attention + FFN, final LN, mean-pool head.

Sharding: data-parallel over batch. B=16 across 8 cores -> 2 batch elements/core.
Each core runs the full model on its 2 elements; no collectives.

On-chip layout convention:
  - "feature-major" activations: [feat(partitions, 4x128), token(free, 512)]
  - "token-major"   activations: [token(partitions, 4x128), feat(free)]
  Projections q/k are produced feature-major (ready as matmul lhsT for scores);
  v is produced token-major (ready as AV rhs). Attention probs are transposed on
  the PE (identity matmul) so AV contracts over keys on partitions.
"""
import numpy as np
from contextlib import ExitStack

# ---------------------------------------------------------------- constants
P = 128
D = 512          # token dim
L = 512          # tokens per batch element
H = 8            # heads
E = 64           # head dim
DFF = 2048
NLAYER = 3
NB = 2           # batch elements per core
NCORES = 8
W_SIZES = (8, 16, 32, 64, 96, 128, 192, 256)
CONV_O = 447     # conv output channels (= 512 - 2*32 - 1)
IN_C = 32
L_PATCH = 16
MASK_VAL = -1e9
FO = D // P      # 4 feature tiles
TO = L // P      # 4 token tiles

_PROG_CACHE = {}


def _band(h, t):
    """s-tile range (inclusive lo, count) for head h, token-tile t."""
    r = 1 if W_SIZES[h] <= P else 2
    s_lo = max(0, t - r)
    s_hi = min(TO - 1, t + r)
    return s_lo, s_hi - s_lo + 1


# ---------------------------------------------------------------- program
def _build_program(nb=NB, nl=NLAYER):
    import concourse.bass as bass
    import concourse.tile as tile
    from concourse import bacc, mybir
    from concourse.masks import make_identity

    f32 = mybir.dt.float32
    bf16 = mybir.dt.bfloat16
    Alu = mybir.AluOpType
    Act = mybir.ActivationFunctionType

    nc = bacc.Bacc("TRN2", target_bir_lowering=False, debug=False)

    # ---------------- DRAM parameters (per-core views; weights replicated)
    d = {}

    def inp(name, shape):
        d[name] = nc.dram_tensor(name, list(shape), f32, kind="ExternalInput")
        return d[name]

    inp("x3", (nb, L, D))                 # x reshaped [b, n, h*32+i]
    inp("tokWcat", (D, CONV_O + IN_C))    # [k=(h,i), conv 447 | mean 32]
    inp("tokb", (D,))                     # conv bias padded to 512
    inp("pos", (D,))                      # sin(0.1*n)
    inp("wqT", (nl, D, D))                # pre-scaled by 1/8, transposed
    inp("wkT", (nl, D, D))
    inp("wvT", (nl, D, D))
    inp("woT", (nl, D, D))
    inp("bq", (nl, D))                    # pre-scaled by 1/8
    inp("bk", (nl, D))
    inp("bv", (nl, D))
    inp("bo", (nl, D))
    inp("w1T", (nl, D, DFF))
    inp("b1", (nl, DFF))
    inp("w2T", (nl, DFF, D))
    inp("b2", (nl, D))
    inp("g1", (nl, D))
    inp("be1", (nl, D))
    inp("g2", (nl, D))
    inp("be2", (nl, D))
    inp("lnf_g", (D,))
    inp("lnf_b", (D,))
    inp("headwT", (D, 16))                # head_w.T padded 10->16
    inp("headb", (16,))

    enc_out = nc.dram_tensor("enc_out", [nb, L, D], f32, kind="ExternalOutput")
    y_out = nc.dram_tensor("y_out", [nb, 16], f32, kind="ExternalOutput")

    def bcast(ap, parts, n):
        return bass.AP(tensor=ap.tensor, offset=ap.offset, ap=[[0, parts], [1, n]])

    def col(ap, n):  # 1-D dram vector [n] -> [n, 1]
        return bass.AP(tensor=ap.tensor, offset=ap.offset, ap=[[1, n], [1, 1]])

    with tile.TileContext(nc) as tc, ExitStack() as ctx:
        const = ctx.enter_context(tc.tile_pool(name="const", bufs=1))
        wpool = ctx.enter_context(tc.tile_pool(name="wpool", bufs=1))
        wchunk = ctx.enter_context(tc.tile_pool(name="wchunk", bufs=1))
        bias_p = ctx.enter_context(tc.tile_pool(name="bias", bufs=1))
        hpool = ctx.enter_context(tc.tile_pool(name="hpool", bufs=2))
        qkv_p = ctx.enter_context(tc.tile_pool(name="qkv", bufs=1))
        work = ctx.enter_context(tc.tile_pool(name="work", bufs=1))
        soft = ctx.enter_context(tc.tile_pool(name="soft", bufs=2))
        attn_p = ctx.enter_context(tc.tile_pool(name="attn", bufs=1))
        lnp = ctx.enter_context(tc.tile_pool(name="ln", bufs=2))
        small = ctx.enter_context(tc.tile_pool(name="small", bufs=8))
        ps = ctx.enter_context(tc.tile_pool(name="ps", bufs=1, space="PSUM"))
        ps_att = ctx.enter_context(tc.tile_pool(name="ps_att", bufs=1, space="PSUM"))
        ps_ff = ctx.enter_context(tc.tile_pool(name="ps_ff", bufs=2, space="PSUM"))

        # ---------------- constants
        ident = const.tile([P, P], f32, tag="ident")
        make_identity(nc, ident)
        eps_t = const.tile([P, 1], f32, tag="eps")
        nc.vector.memset(eps_t, 1e-5)
        ones_col = const.tile([P, 1], f32, tag="ones")
        nc.vector.memset(ones_col, 1.0)

        # band masks, bf16, ONE tile per head covering the full (2r+1)-block band
        # pattern; per (h, t) usage takes a column slice (the pattern repeats).
        head_mask = {}
        for h in range(H):
            r = 1 if W_SIZES[h] <= P else 2
            mw = (2 * r + 1) * P
            off = r * P
            w = W_SIZES[h]
            m = const.tile([P, mw], bf16, tag=f"mask{h}", name=f"mask{h}")
            nc.gpsimd.memset(m, 0.0)
            # keep where (l - s + w) >= 0, i.e. p - j + off + w >= 0
            nc.gpsimd.affine_select(
                out=m, in_=m, compare_op=Alu.is_ge, fill=MASK_VAL,
                base=off + w, pattern=[[-1, mw]], channel_multiplier=1)
            # keep where (s - l + w) >= 0, i.e. -p + j - off + w >= 0
            nc.gpsimd.affine_select(
                out=m, in_=m, compare_op=Alu.is_ge, fill=MASK_VAL,
                base=w - off, pattern=[[1, mw]], channel_multiplier=-1)
            head_mask[h] = m

        def mask_slice(h, t):
            s_lo, nblk = _band(h, t)
            r = 1 if W_SIZES[h] <= P else 2
            lo = (r - (t - s_lo)) * P      # interior offset r*128 minus actual offset
            return head_mask[h][:, lo:lo + nblk * P]

        tokW_sb = const.tile([P, FO, CONV_O + IN_C], f32, tag="tokW")
        nc.sync.dma_start(out=tokW_sb, in_=d["tokWcat"].ap().rearrange("(o p) m -> p o m", p=P))
        tokb_sb = const.tile([P, FO], f32, tag="tokb")
        nc.sync.dma_start(out=tokb_sb, in_=d["tokb"].ap().rearrange("(o p) -> p o", p=P))
        lnfg_bc = const.tile([P, D], f32, tag="lnfg")
        nc.sync.dma_start(out=lnfg_bc, in_=bcast(d["lnf_g"].ap(), P, D))
        lnfb_bc = const.tile([P, D], f32, tag="lnfb")
        nc.sync.dma_start(out=lnfb_bc, in_=bcast(d["lnf_b"].ap(), P, D))
        headw_sb = const.tile([P, FO, 16], f32, tag="headw")
        nc.sync.dma_start(out=headw_sb, in_=d["headwT"].ap().rearrange("(o p) m -> p o m", p=P))
        headb_sb = const.tile([16, 1], f32, tag="headb")
        nc.sync.dma_start(out=headb_sb, in_=col(d["headb"].ap(), 16))

        # ---------------- helpers
        def emit_tokenize(b, hT):
            # token-major patches; reuse qkv-pool slots (free before layer 0)
            xt = qkv_p.tile([P, TO, D], f32, tag="qT", name="xt")
            nc.sync.dma_start(out=xt, in_=d["x3"].ap()[b].rearrange("(o p) m -> p o m", p=P))
            xT = qkv_p.tile([P, FO, L], f32, tag="kT", name="xT")
            xSq = qkv_p.tile([P, FO, L], f32, tag="v", name="xSq")
            for j in range(FO):
                tp = ps.tile([P, 512], f32, tag="tp", name="tp")
                for i in range(TO):
                    nc.tensor.transpose(
                        tp[:, i * P:(i + 1) * P], xt[:, i, j * P:(j + 1) * P], ident)
                nc.vector.tensor_copy(xT[:, j, :], tp)
                nc.scalar.activation(out=xSq[:, j, :], in_=tp, func=Act.Square)
            # conv + mean via tokWcat matmuls; m-tiles 0..2 full, 3 partial (conv 63)
            for mo in range(FO):
                mm = ps.tile([P, 512], f32, tag="mm", name="mm", bufs=2)
                msz = 128 if mo < 3 else CONV_O - 384
                for k in range(FO):
                    nc.tensor.matmul(
                        mm[:msz, :], tokW_sb[:, k, mo * P:mo * P + msz], xT[:, k, :],
                        start=(k == 0), stop=(k == FO - 1))
                nc.scalar.activation(
                    out=hT[:msz, mo, :], in_=mm[:msz, :], func=Act.Identity,
                    bias=tokb_sb[:msz, mo:mo + 1])
            # mean / meansq [32, 512]
            mean_ps = ps.tile([P, 512], f32, tag="mm", name="mean_ps", bufs=2)
            for k in range(FO):
                nc.tensor.matmul(mean_ps[:IN_C, :], tokW_sb[:, k, CONV_O:], xT[:, k, :],
                                 start=(k == 0), stop=(k == FO - 1))
            mean_sb = lnp.tile([IN_C, L], f32, tag="mean_sb", name="mean_sb", bufs=1)
            nc.vector.tensor_copy(mean_sb, mean_ps[:IN_C, :])
            ms_ps = ps.tile([P, 512], f32, tag="mm", name="ms_ps", bufs=2)
            for k in range(FO):
                nc.tensor.matmul(ms_ps[:IN_C, :], tokW_sb[:, k, CONV_O:], xSq[:, k, :],
                                 start=(k == 0), stop=(k == FO - 1))
            # std = sqrt(meansq - mean^2)
            var_sb = lnp.tile([IN_C, L], f32, tag="var_sb", name="var_sb", bufs=1)
            nc.vector.tensor_mul(var_sb, mean_sb, mean_sb)
            nc.vector.tensor_sub(var_sb, ms_ps[:IN_C, :], var_sb)
            nc.scalar.activation(out=var_sb, in_=var_sb, func=Act.Sqrt)
            # place mean/std/pos rows into hT m-tile 3 (partition-shifting DMAs)
            nc.sync.dma_start(out=hT[63:95, 3, :], in_=mean_sb)
            nc.sync.dma_start(out=hT[95:127, 3, :], in_=var_sb)
            nc.sync.dma_start(out=hT[127:128, 3, :], in_=d["pos"].ap()[None, :])

        def emit_qkv(hT, qT, kT, v_sb, wq_sb, wk_sb, wv_sb, bq_sb, bk_sb, bv_bc):
            for mo in range(FO):
                q_ps = ps.tile([P, 512], f32, tag="mm", name="q_ps", bufs=2)
                for k in range(FO):
                    nc.tensor.matmul(q_ps, wq_sb[:, k, mo * P:(mo + 1) * P], hT[:, k, :],
                                     start=(k == 0), stop=(k == FO - 1))
                nc.scalar.activation(out=qT[:, mo, :], in_=q_ps, func=Act.Identity,
                                     bias=bq_sb[:, mo:mo + 1])
                k_ps = ps.tile([P, 512], f32, tag="mm", name="k_ps", bufs=2)
                for k in range(FO):
                    nc.tensor.matmul(k_ps, wk_sb[:, k, mo * P:(mo + 1) * P], hT[:, k, :],
                                     start=(k == 0), stop=(k == FO - 1))
                nc.scalar.activation(out=kT[:, mo, :], in_=k_ps, func=Act.Identity,
                                     bias=bk_sb[:, mo:mo + 1])
                v_ps = ps.tile([P, 512], f32, tag="mm", name="v_ps", bufs=2)
                for k in range(FO):
                    nc.tensor.matmul(v_ps, hT[:, k, mo * P:(mo + 1) * P], wv_sb[:, k, :],
                                     start=(k == 0), stop=(k == FO - 1))
                nc.vector.tensor_add(v_sb[:, mo, :], v_ps, bv_bc)

        def emit_attention(qT, kT, v_sb, attn_t):
            """attn_t: list of 4 token-major tiles [128, 512] (normalized heads)."""
            for t in range(TO):
                out_ps = ps_att.tile([P, 512], f32, tag="outacc", name="out_ps")
                sums = small.tile([P, H], f32, tag="sums", name="sums")
                for h in range(H):
                    s_lo, nblk = _band(h, t)
                    width = nblk * P
                    mo, pb = h // 2, (h % 2) * E
                    sc = ps_att.tile([P, 512], f32, tag="sc", name="sc", bufs=1)
                    nc.tensor.matmul(
                        sc[:, :width],
                        qT[pb:pb + E, mo, t * P:(t + 1) * P],
                        kT[pb:pb + E, mo, s_lo * P:s_lo * P + width],
                        start=True, stop=True)
                    am = soft.tile([P, 512], f32, tag="aneg", name="am", bufs=1)
                    nc.vector.tensor_add(am[:, :width], sc[:, :width], mask_slice(h, t))
                    negmax = small.tile([P, 1], f32, tag="negmax", name="negmax")
                    nc.vector.tensor_reduce(
                        out=negmax, in_=am[:, :width], axis=mybir.AxisListType.X,
                        op=Alu.max, negate=True)
                    aexp = soft.tile([P, 512], f32, tag="aexp", name="aexp")
                    nc.scalar.activation(
                        out=aexp[:, :width], in_=am[:, :width], func=Act.Exp,
                        bias=negmax, scale=1.0, accum_out=sums[:, h:h + 1])
                    at_ps = ps_att.tile([P, 512], f32, tag="at", name="at_ps", bufs=1)
                    for bi in range(nblk):
                        nc.tensor.transpose(
                            at_ps[:, bi * P:(bi + 1) * P], aexp[:, bi * P:(bi + 1) * P], ident)
                    at_sb = soft.tile([P, 512], f32, tag="atsb", name="at_sb")
                    nc.vector.tensor_copy(at_sb[:, :width], at_ps[:, :width])
                    for bi in range(nblk):
                        nc.tensor.matmul(
                            out_ps[:, h * E:(h + 1) * E],
                            at_sb[:, bi * P:(bi + 1) * P],
                            v_sb[:, s_lo + bi, h * E:(h + 1) * E],
                            start=(bi == 0), stop=(bi == nblk - 1))
                rsum = small.tile([P, H], f32, tag="rsum", name="rsum")
                nc.vector.reciprocal(rsum, sums)
                for h in range(H):
                    nc.vector.tensor_scalar_mul(
                        out=attn_t[t][:, h * E:(h + 1) * E],
                        in0=out_ps[:, h * E:(h + 1) * E], scalar1=rsum[:, h:h + 1])

        def emit_ln(x_feat, out_feat, g_sb, b_sb, xn_tag):
            """LayerNorm over features. x_feat/out_feat: feature-major [128,4,512]."""
            xn_tiles = []
            for t in range(TO):
                tp = ps.tile([P, 512], f32, tag="tp", name="tp")
                for j in range(FO):
                    nc.tensor.transpose(
                        tp[:, j * P:(j + 1) * P], x_feat[:, j, t * P:(t + 1) * P], ident)
                xtok = lnp.tile([P, D], f32, tag="xtok", name="xtok")
                nc.vector.tensor_copy(xtok, tp)
                stats = small.tile([P, 6], f32, tag="stats", name="stats")
                nc.vector.bn_stats(out=stats, in_=xtok)
                mv = small.tile([P, 2], f32, tag="mv", name="mv")
                nc.vector.bn_aggr(out=mv, in_=stats)
                sd = small.tile([P, 1], f32, tag="sd", name="sd")
                nc.scalar.activation(out=sd, in_=mv[:, 1:2], func=Act.Sqrt, bias=eps_t)
                rstd = small.tile([P, 1], f32, tag="rstd", name="rstd")
                nc.vector.reciprocal(rstd, sd)
                xn = attn_p.tile([P, D], f32, tag=f"{xn_tag}{t}", name="xn")
                nc.vector.tensor_scalar(
                    out=xn, in0=xtok, scalar1=mv[:, 0:1], scalar2=rstd,
                    op0=Alu.subtract, op1=Alu.mult)
                xn_tiles.append(xn)
            for j in range(FO):
                tp2 = ps.tile([P, 512], f32, tag="tp", name="tp2")
                for t in range(TO):
                    nc.tensor.transpose(
                        tp2[:, t * P:(t + 1) * P], xn_tiles[t][:, j * P:(j + 1) * P], ident)
                nc.vector.tensor_scalar(
                    out=out_feat[:, j, :], in0=tp2, scalar1=g_sb[:, j:j + 1],
                    scalar2=b_sb[:, j:j + 1], op0=Alu.mult, op1=Alu.add)

        def emit_layer(l, b, hT, wsb):
            (wq_sb, wk_sb, wv_sb, wo_sb, bq_sb, bk_sb, bv_bc, bo_sb,
             b1_sb, b2_sb, g1_sb, be1_sb, g2_sb, be2_sb) = wsb
            qT = qkv_p.tile([P, FO, L], f32, tag="qT", name="qT")
            kT = qkv_p.tile([P, FO, L], f32, tag="kT", name="kT")
            v_sb = qkv_p.tile([P, FO, D], f32, tag="v", name="v_sb")
            emit_qkv(hT, qT, kT, v_sb, wq_sb, wk_sb, wv_sb, bq_sb, bk_sb, bv_bc)
            attn_t = [attn_p.tile([P, D], f32, tag=f"attn_t{t}", name=f"attn_t{t}")
                      for t in range(TO)]
            emit_attention(qT, kT, v_sb, attn_t)
            # transpose attn to feature-major
            attnT = work.tile([P, FO, L], f32, tag="attnT", name="attnT")
            for j in range(FO):
                tp = ps.tile([P, 512], f32, tag="tp", name="tp")
                for t in range(TO):
                    nc.tensor.transpose(
                        tp[:, t * P:(t + 1) * P], attn_t[t][:, j * P:(j + 1) * P], ident)
                nc.vector.tensor_copy(attnT[:, j, :], tp)
            # O projection + residual -> x1
            x1 = work.tile([P, FO, L], f32, tag="x1", name="x1")
            for mo in range(FO):
                o_ps = ps.tile([P, 512], f32, tag="mm", name="o_ps", bufs=2)
                for k in range(FO):
                    nc.tensor.matmul(o_ps, wo_sb[:, k, mo * P:(mo + 1) * P], attnT[:, k, :],
                                     start=(k == 0), stop=(k == FO - 1))
                nc.vector.scalar_tensor_tensor(
                    out=x1[:, mo, :], in0=o_ps, scalar=bo_sb[:, mo:mo + 1],
                    in1=hT[:, mo, :], op0=Alu.add, op1=Alu.add)
            # LN1
            x1ln = work.tile([P, FO, L], f32, tag="x1ln", name="x1ln")
            emit_ln(x1, x1ln, g1_sb, be1_sb, "attn_t")
            # FFN (dff chunks of 512); accumulate output chunks in SBUF (x2)
            x2 = hpool.tile([P, FO, L], f32, tag=f"hT{b}", name="x2")
            n_ch = DFF // 512
            for c in range(n_ch):
                w1c = wchunk.tile([P, FO, 512], f32, tag="w1c", name="w1c")
                nc.sync.dma_start(
                    out=w1c,
                    in_=d["w1T"].ap()[l].rearrange("(o p) m -> p o m", p=P)[:, :, c * 512:(c + 1) * 512])
                w2c = wchunk.tile([P, FO, D], f32, tag="w2c", name="w2c")
                nc.sync.dma_start(
                    out=w2c,
                    in_=d["w2T"].ap()[l][c * 512:(c + 1) * 512, :].rearrange("(o p) m -> p o m", p=P))
                y1c = work.tile([P, FO, 512], f32, tag="attnT", name="y1c")
                for mo in range(FO):
                    y1_ps = ps_ff.tile([P, 512], f32, tag="ff", name="y1_ps")
                    for k in range(FO):
                        nc.tensor.matmul(
                            y1_ps, w1c[:, k, mo * P:(mo + 1) * P], x1ln[:, k, :],
                            start=(k == 0), stop=(k == FO - 1))
                    nc.scalar.activation(
                        out=y1c[:, mo, :], in_=y1_ps, func=Act.Gelu,
                        bias=b1_sb[:, (c * FO + mo):(c * FO + mo + 1)])
                for mo in range(FO):
                    y_ps = ps_ff.tile([P, 512], f32, tag="ff", name="y_ps")
                    for k in range(FO):
                        nc.tensor.matmul(
                            y_ps, w2c[:, k, mo * P:(mo + 1) * P], y1c[:, k, :],
                            start=(k == 0), stop=(k == FO - 1))
                    if c == 0:
                        nc.vector.scalar_tensor_tensor(
                            out=x2[:, mo, :], in0=y_ps, scalar=b2_sb[:, mo:mo + 1],
                            in1=x1ln[:, mo, :], op0=Alu.add, op1=Alu.add)
                    else:
                        nc.vector.tensor_add(x2[:, mo, :], y_ps, x2[:, mo, :])
            # LN2 -> next h
            h_next = hpool.tile([P, FO, L], f32, tag=f"hT{b}", name="h_next")
            emit_ln(x2, h_next, g2_sb, be2_sb, "attn_t")
            return h_next

        def emit_final(b, hT):
            # final LN in token-major; write enc; pool + head
            pool_ps = ps.tile([P, 512], f32, tag="mm", name="pool_ps", bufs=2)
            for t in range(TO):
                tp = ps.tile([P, 512], f32, tag="tp", name="tp")
                for j in range(FO):
                    nc.tensor.transpose(
                        tp[:, j * P:(j + 1) * P], hT[:, j, t * P:(t + 1) * P], ident)
                xtok = lnp.tile([P, D], f32, tag="xtok", name="xtok")
                nc.vector.tensor_copy(xtok, tp)
                stats = small.tile([P, 6], f32, tag="stats", name="stats")
                nc.vector.bn_stats(out=stats, in_=xtok)
                mv = small.tile([P, 2], f32, tag="mv", name="mv")
                nc.vector.bn_aggr(out=mv, in_=stats)
                sd = small.tile([P, 1], f32, tag="sd", name="sd")
                nc.scalar.activation(out=sd, in_=mv[:, 1:2], func=Act.Sqrt, bias=eps_t)
                rstd = small.tile([P, 1], f32, tag="rstd", name="rstd")
                nc.vector.reciprocal(rstd, sd)
                xn = lnp.tile([P, D], f32, tag="xn_f", name="xn")
                nc.vector.tensor_scalar(
                    out=xn, in0=xtok, scalar1=mv[:, 0:1], scalar2=rstd,
                    op0=Alu.subtract, op1=Alu.mult)
                enc_t = lnp.tile([P, D], f32, tag="enc_t", name="enc_t")
                nc.vector.tensor_mul(enc_t, xn, lnfg_bc)
                nc.vector.tensor_add(enc_t, enc_t, lnfb_bc)
                nc.sync.dma_start(out=enc_out.ap()[b][t * P:(t + 1) * P, :], in_=enc_t)
                nc.tensor.matmul(pool_ps[:1, :], ones_col, enc_t,
                                 start=(t == 0), stop=(t == TO - 1))
            pool_sb = small.tile([1, D], f32, tag="pool_sb", name="pool_sb", bufs=1)
            nc.scalar.mul(pool_sb, pool_ps[:1, :], 1.0 / L)
            pt_ps = ps.tile([P, 512], f32, tag="tp", name="pt_ps")
            for k in range(FO):
                nc.tensor.transpose(pt_ps[:, k:k + 1], pool_sb[0:1, k * P:(k + 1) * P],
                                    ident[0:1, 0:1])
            poolT = small.tile([P, FO], f32, tag="poolT", name="poolT", bufs=1)
            nc.vector.tensor_copy(poolT, pt_ps[:, :FO])
            y_ps = ps.tile([P, 512], f32, tag="mm", name="y_ps", bufs=2)
            for k in range(FO):
                nc.tensor.matmul(y_ps[:16, :1], headw_sb[:, k, :], poolT[:, k:k + 1],
                                 start=(k == 0), stop=(k == FO - 1))
            y_sb = small.tile([16, 1], f32, tag="y_sb", name="y_sb", bufs=1)
            nc.vector.tensor_add(y_sb, y_ps[:16, :1], headb_sb)
            nc.sync.dma_start(out=y_out.ap()[b].rearrange("(p o) -> p o", o=1), in_=y_sb)

        # ---------------- emit the full program
        hts = []
        for b in range(nb):
            hT = hpool.tile([P, FO, L], f32, tag=f"hT{b}", name=f"hT{b}")
            emit_tokenize(b, hT)
            hts.append(hT)

        for l in range(nl):
            wq_sb = wpool.tile([P, FO, D], f32, tag="wqT", name="wq_sb")
            nc.sync.dma_start(out=wq_sb, in_=d["wqT"].ap()[l].rearrange("(o p) m -> p o m", p=P))
            wk_sb = wpool.tile([P, FO, D], f32, tag="wkT", name="wk_sb")
            nc.sync.dma_start(out=wk_sb, in_=d["wkT"].ap()[l].rearrange("(o p) m -> p o m", p=P))
            wv_sb = wpool.tile([P, FO, D], f32, tag="wvT", name="wv_sb")
            nc.sync.dma_start(out=wv_sb, in_=d["wvT"].ap()[l].rearrange("(o p) m -> p o m", p=P))
            wo_sb = wpool.tile([P, FO, D], f32, tag="woT", name="wo_sb")
            nc.sync.dma_start(out=wo_sb, in_=d["woT"].ap()[l].rearrange("(o p) m -> p o m", p=P))
            bq_sb = bias_p.tile([P, FO], f32, tag="bq", name="bq_sb", bufs=2)
            nc.sync.dma_start(out=bq_sb, in_=d["bq"].ap()[l].rearrange("(o p) -> p o", p=P))
            bk_sb = bias_p.tile([P, FO], f32, tag="bk", name="bk_sb", bufs=2)
            nc.sync.dma_start(out=bk_sb, in_=d["bk"].ap()[l].rearrange("(o p) -> p o", p=P))
            bv_bc = bias_p.tile([P, D], f32, tag="bv", name="bv_bc", bufs=2)
            nc.sync.dma_start(out=bv_bc, in_=bcast(d["bv"].ap()[l], P, D))
            bo_sb = bias_p.tile([P, FO], f32, tag="bo", name="bo_sb", bufs=2)
            nc.sync.dma_start(out=bo_sb, in_=d["bo"].ap()[l].rearrange("(o p) -> p o", p=P))
            b1_sb = bias_p.tile([P, DFF // P], f32, tag="b1", name="b1_sb", bufs=2)
            nc.sync.dma_start(out=b1_sb, in_=d["b1"].ap()[l].rearrange("(o p) -> p o", p=P))
            b2_sb = bias_p.tile([P, FO], f32, tag="b2", name="b2_sb", bufs=2)
            nc.sync.dma_start(out=b2_sb, in_=d["b2"].ap()[l].rearrange("(o p) -> p o", p=P))
            g1_sb = bias_p.tile([P, FO], f32, tag="g1", name="g1_sb", bufs=2)
            nc.sync.dma_start(out=g1_sb, in_=d["g1"].ap()[l].rearrange("(o p) -> p o", p=P))
            be1_sb = bias_p.tile([P, FO], f32, tag="be1", name="be1_sb", bufs=2)
            nc.sync.dma_start(out=be1_sb, in_=d["be1"].ap()[l].rearrange("(o p) -> p o", p=P))
            g2_sb = bias_p.tile([P, FO], f32, tag="g2", name="g2_sb", bufs=2)
            nc.sync.dma_start(out=g2_sb, in_=d["g2"].ap()[l].rearrange("(o p) -> p o", p=P))
            be2_sb = bias_p.tile([P, FO], f32, tag="be2", name="be2_sb", bufs=2)
            nc.sync.dma_start(out=be2_sb, in_=d["be2"].ap()[l].rearrange("(o p) -> p o", p=P))
            wsb = (wq_sb, wk_sb, wv_sb, wo_sb, bq_sb, bk_sb, bv_bc, bo_sb,
                   b1_sb, b2_sb, g1_sb, be1_sb, g2_sb, be2_sb)
            for b in range(nb):
                hts[b] = emit_layer(l, b, hts[b], wsb)

        for b in range(nb):
            emit_final(b, hts[b])

    nc.compile()
    return nc


# ---------------------------------------------------------------- host side
def _prep_shared(inputs, nl=NLAYER):
    f = np.float32
    s = {}
    tok_w = np.asarray(inputs["tok_w"], f)          # [447, 32, 16] (O, I, H)
    tokWcat = np.zeros((D, CONV_O + IN_C), f)
    tokWcat[:, :CONV_O] = tok_w.transpose(2, 1, 0).reshape(D, CONV_O)
    for c in range(IN_C):
        tokWcat[np.arange(L_PATCH) * IN_C + c, CONV_O + c] = 1.0 / L_PATCH
    s["tokWcat"] = tokWcat
    tokb = np.zeros(D, f)
    tokb[:CONV_O] = np.asarray(inputs["tok_b"], f)
    s["tokb"] = tokb
    s["pos"] = np.sin(np.arange(L, dtype=f) * f(0.1)).astype(f)
    sc = f(0.125)  # 1/sqrt(E)
    s["wqT"] = np.ascontiguousarray(np.asarray(inputs["wq"], f).transpose(0, 2, 1)) * sc
    s["wkT"] = np.ascontiguousarray(np.asarray(inputs["wk"], f).transpose(0, 2, 1))
    s["wvT"] = np.ascontiguousarray(np.asarray(inputs["wv"], f).transpose(0, 2, 1))
    s["woT"] = np.ascontiguousarray(np.asarray(inputs["wo"], f).transpose(0, 2, 1))
    s["bq"] = np.asarray(inputs["bq"], f) * sc
    s["bk"] = np.asarray(inputs["bk"], f)
    s["bv"] = np.asarray(inputs["bv"], f)
    s["bo"] = np.asarray(inputs["bo"], f)
    s["w1T"] = np.ascontiguousarray(np.asarray(inputs["w1"], f).transpose(0, 2, 1))
    s["b1"] = np.asarray(inputs["b1"], f)
    s["w2T"] = np.ascontiguousarray(np.asarray(inputs["w2"], f).transpose(0, 2, 1))
    s["b2"] = np.asarray(inputs["b2"], f)
    s["g1"] = np.asarray(inputs["ln1_g"], f)
    s["be1"] = np.asarray(inputs["ln1_b"], f)
    s["g2"] = np.asarray(inputs["ln2_g"], f)
    s["be2"] = np.asarray(inputs["ln2_b"], f)
    s["lnf_g"] = np.asarray(inputs["lnf_g"], f)
    s["lnf_b"] = np.asarray(inputs["lnf_b"], f)
    headwT = np.zeros((D, 16), f)
    headwT[:, :10] = np.asarray(inputs["head_w"], f).T
    s["headwT"] = headwT
    headb = np.zeros(16, f)
    headb[:10] = np.asarray(inputs["head_b"], f)
    s["headb"] = headb
    if nl < NLAYER:
        for k in ("wqT", "wkT", "wvT", "woT", "bq", "bk", "bv", "bo",
                  "w1T", "b1", "w2T", "b2", "g1", "be1", "g2", "be2"):
            s[k] = np.ascontiguousarray(s[k][:nl])
    return s


def _run(inputs, trace=False):
    from concourse.bass_utils import run_bass_kernel_spmd

    key = (NB, NLAYER)
    if key not in _PROG_CACHE:
        _PROG_CACHE[key] = _build_program(*key)
    nc = _PROG_CACHE[key]

    shared = _prep_shared(inputs)
    x = np.asarray(inputs["x"], np.float32)          # [16, 8192, 32]
    B = x.shape[0]
    per = B // NCORES
    in_maps = []
    for i in range(NCORES):
        m = dict(shared)
        m["x3"] = np.ascontiguousarray(x[i * per:(i + 1) * per].reshape(per, L, D))
        in_maps.append(m)

    res = run_bass_kernel_spmd(nc, in_maps, list(range(NCORES)), trace=trace)
    enc = np.concatenate([res.results[i]["enc_out"] for i in range(NCORES)], axis=0)
    y = np.concatenate([res.results[i]["y_out"][:, :10] for i in range(NCORES)], axis=0)
    return (y, enc), res


def kernel(**inputs):
    (y, enc), _ = _run(inputs)
    return (y, enc)
